# revision 7
# baseline (speedup 1.0000x reference)
"""LocallyConnected2d kernel for 8 TRN2 NeuronCores (Bass/Tile).

Problem (hardcoded):
  features [32, 64, 64, 64] f32, weights [62, 62, 64, 64, 3, 3] f32,
  bias [62, 62, 64] f32 -> out [32, 64, 62, 62] f32
  out[b,o,h,w] = sum_{c,i,j} x[b,c,h+i,w+j] * W[h,w,o,c,i,j] + bias[h,w,o]

Strategy:
  - Shard over Hout: 8 cores x 8 output rows (bands [0,8,...,48,54], the last
    two overlap; host takes canonical rows from each core).
  - bf16 on the PE, fp32 PSUM accumulate. Single dual-shifted feature tile
    fA (p<64: x[c,t,w,b]; p>=64: x[c,t,w+1,b]) so a [128,32] AP slice pairs
    kernel cols (d=0,d=1) in one K=128 matmul; kernel col d=2 is covered by
    K=64 matmuls on the lower half of fA sliced at w+2 -- no second feature
    tile, saving its 5.2MB of DMA-engine traffic vs the fB variant.
  - Work unit = (half-band hg, group of 4 w): PSUM tile [128,256] with
    partitions=(4w x 32b) via col tile_position and free=(4 output rows x 64
    cout). ONE accumulation group per tile (single start=True zeroing matmul;
    per-element has_written gives overwrite-on-first-touch).
  - Matmuls grouped by stationary: a patch at absolute row t serves all
    (out-row j, kernel-row r) with j+r=t-hl in ONE matmul with a wide moving
    operand (weights host-concatenated, N up to 192). Same t-grouping for the
    d=2 (K=64) matmuls, whose weights ride a separate 64-partition stream
    (all matmul operands stay on base partition 0 -- row-64 PE tiles fault).
  - The 16 SDMA engines (~26 GB/s each, ~410 GB/s aggregate) are shared by
    all queues; weights are split over BOTH HWDGE rings in two-group
    transfers with wr/wd2 pair parity opposed (sync: even wr + odd wd2,
    scalar: odd wr + even wd2) so per-transfer sync-point bubbles on one
    queue are hidden by the other. featA rides the gpsimd SWDGE ring.
    Output stored in 256KB chunks as each 4-group slab of S is ready.
  - Host: shard/pack inputs, unpack outS dumps, add bias, assemble f32 out.
"""

import numpy as np
import ml_dtypes

BF16 = ml_dtypes.bfloat16

B, CIN, COUT = 32, 64, 64
H = W = 64
HOUT = WOUT = 62
NCORES = 8
STARTS = [0, 8, 16, 24, 32, 40, 48, 54]

# t-group geometry: tau = t - hl in 0..5; valid out-rows j in [jlo, jhi]
TAUS = list(range(6))
JLO = [max(0, t - 2) for t in TAUS]
JHI = [min(3, t) for t in TAUS]
NV = [hi - lo + 1 for lo, hi in zip(JLO, JHI)]          # [1,2,3,3,2,1]
TBASE = [0]
for t in TAUS:
    TBASE.append(TBASE[-1] + 4 * NV[t] * 64)            # per-(tau) base col
WR_COLS = TBASE[-1]                                      # 3072

_STATE = {}


def _build_program():
    import concourse.tile as tile
    from concourse import bacc, mybir

    bf = mybir.dt.bfloat16
    f32 = mybir.dt.float32

    nc = bacc.Bacc(None, target_bir_lowering=False)
    featA = nc.dram_tensor("featA", [128, 10, 64, 32], bf, kind="ExternalInput")
    wr_d = nc.dram_tensor("wr", [2, 8, 128, 2, WR_COLS], bf,
                          kind="ExternalInput")
    wd2_d = nc.dram_tensor("wd2", [2, 8, 64, 2, WR_COLS], bf,
                           kind="ExternalInput")
    outS = nc.dram_tensor("outS", [2, 128, 4096], bf, kind="ExternalOutput")

    with tile.TileContext(nc) as tc:
        with tc.tile_pool(name="feat", bufs=1) as fpool, \
             tc.tile_pool(name="wr", bufs=4) as wrpool, \
             tc.tile_pool(name="wd2", bufs=4) as wdpool, \
             tc.tile_pool(name="st", bufs=2) as spool, \
             tc.tile_pool(name="ps", bufs=8, space="PSUM") as pspool:
            # featA on the gpsimd SWDGE ring (both HWDGE rings carry weights),
            # row-chunked so hg=0 matmuls unblock on the first chunk.
            fA = fpool.tile([128, 10, 64, 32], bf)
            nc.scalar.dma_start(fA[:, 0:6], featA[:, 0:6])
            nc.scalar.dma_start(fA[:, 6:10], featA[:, 6:10])
            # zero operands for the psum-clearing matmul (see below)
            zl = fpool.tile([1, 128], bf)
            nc.gpsimd.memset(zl[:], 0.0)
            zr = fpool.tile([1, 256], bf)
            nc.gpsimd.memset(zr[:], 0.0)
            for hg in range(2):
                hl = 4 * hg
                S = spool.tile([128, 4096], bf)
                for pr in range(8):
                    wr = wrpool.tile([128, 2, WR_COLS], bf)
                    wd2 = wdpool.tile([64, 2, WR_COLS], bf)
                    nc.sync.dma_start(wr[:], wr_d[hg, pr])
                    nc.sync.dma_start(wd2[:], wd2_d[hg, pr])
                    for sub in range(2):
                        wg = 2 * pr + sub
                        w0 = min(4 * wg, 58)  # last group overlaps: w 58..61
                        ps = pspool.tile([128, 256], f32)
                        # K=1 zeroing matmul over the WHOLE tile: starts the
                        # accumulation group, zeroes every element, and
                        # (because its output overlaps all later MMs) forces
                        # the scheduler to keep it first; all real MMs are
                        # then pure order-free flags=0 accumulates.
                        nc.tensor.matmul(ps[:, :], zl[:], zr[:],
                                         start=True, stop=False,
                                         tile_position=(0, 0))
                        for tau in TAUS:
                            nv, jlo = NV[tau], JLO[tau]
                            for g in range(4):
                                off = TBASE[tau] + g * nv * 64
                                nc.tensor.matmul(
                                    ps[32 * g:32 * g + 32,
                                       64 * jlo:64 * (jlo + nv)],
                                    fA[:, hl + tau, w0 + g, :],
                                    wr[:, sub, off:off + nv * 64],
                                    start=False, stop=False,
                                    tile_position=(0, 32 * g),
                                )
                        # d=2 column: K=64 on the unshifted half at w+2.
                        for tau in TAUS:
                            nv, jlo = NV[tau], JLO[tau]
                            for g in range(4):
                                off = TBASE[tau] + g * nv * 64
                                nc.tensor.matmul(
                                    ps[32 * g:32 * g + 32,
                                       64 * jlo:64 * (jlo + nv)],
                                    fA[0:64, hl + tau, w0 + g + 2, :],
                                    wd2[:, sub, off:off + nv * 64],
                                    start=False, stop=(tau == 5 and g == 3),
                                    tile_position=(0, 32 * g),
                                )
                        nc.vector.tensor_copy(S[:, 256 * wg:256 * wg + 256],
                                              ps[:])
                        if wg % 8 == 7:
                            q = wg // 8
                            nc.scalar.dma_start(
                                outS[hg][:, 2048 * q:2048 * (q + 1)],
                                S[:, 2048 * q:2048 * (q + 1)])
    nc.compile()
    return nc


def _get_nc():
    if "nc" not in _STATE:
        _STATE["nc"] = _build_program()
    return _STATE["nc"]


def _prep_inputs(features, weights):
    """Build the 8 per-core input dicts (bf16, device layouts)."""
    x = np.asarray(features, dtype=np.float32)
    Wt = np.asarray(weights, dtype=np.float32)

    # w-slot -> real w: last group overlaps (w 58..61), no padding needed
    widx = list(range(60)) + [58, 59, 60, 61]

    in_maps = []
    for s in STARTS:
        xt = x[:, :, s:s + 10, :].transpose(1, 2, 3, 0)  # [c, 10, 64, b]
        fA = np.zeros((128, 10, 64, 32), dtype=BF16)
        fA[:64] = xt
        fA[64:, :, :63, :] = xt[:, :, 1:, :]             # w+1 shift

        Wb = Wt[s:s + 8]                                  # [8, 62, o, c, 3, 3]
        Wsel = Wb[:, widx]                                # [8, 64slots, o, c, 3, 3]
        WT = Wsel.transpose(4, 5, 3, 0, 1, 2)             # [i, jw, c, 8h, 64w, o]

        # wr: t-grouped ktiles (cells (r,0)|(r,1)); cols per (tau,g):
        #   q=0..nv-1 -> j=jlo+q, r=tau-j; value(d,c,o)=W[h,w,o,c,r,d]
        wr = np.zeros((2, 16, 128, WR_COLS), dtype=BF16)
        # wd2: t-grouped d=2 cells, K=64; value(c,o)=W[h,w,o,c,r,2]
        wd2 = np.zeros((2, 16, 64, WR_COLS), dtype=BF16)
        for tau in TAUS:
            nv, jlo = NV[tau], JLO[tau]
            view = wr[:, :, :, TBASE[tau]:TBASE[tau + 1]].reshape(
                2, 16, 128, 4, nv, 64)
            view2 = wd2[:, :, :, TBASE[tau]:TBASE[tau + 1]].reshape(
                2, 16, 64, 4, nv, 64)
            for q in range(nv):
                j = jlo + q
                r = tau - j
                for d in range(2):
                    src = WT[r, d].reshape(CIN, 2, 4, 16, 4, COUT)[:, :, j]
                    view[:, :, d * 64:(d + 1) * 64, :, q, :] = \
                        src.transpose(1, 2, 0, 3, 4)      # [hg, wg, c, g, o]
                src = WT[r, 2].reshape(CIN, 2, 4, 16, 4, COUT)[:, :, j]
                view2[:, :, :, :, q, :] = src.transpose(1, 2, 0, 3, 4)
        # two w-groups per transfer: [2, 8pair, P, 2sub, 3072]
        wr = np.ascontiguousarray(
            wr.reshape(2, 8, 2, 128, WR_COLS).transpose(0, 1, 3, 2, 4))
        wd2 = np.ascontiguousarray(
            wd2.reshape(2, 8, 2, 64, WR_COLS).transpose(0, 1, 3, 2, 4))
        in_maps.append({"featA": fA, "wr": wr, "wd2": wd2})
    return in_maps


def _gather(results, bias):
    out = np.zeros((B, COUT, HOUT, WOUT), dtype=np.float32)
    for core, s in enumerate(STARTS):
        arr = np.asarray(results[core]["outS"]).astype(np.float32)
        # [hg, g, b, wg, j, o] -> [b, o, hg, j, wg, g]
        arr = arr.reshape(2, 4, 32, 16, 4, 64).transpose(2, 5, 0, 4, 3, 1)
        arr = arr.reshape(32, 64, 8, 64)
        out[:, :, s:s + 8, 0:60] = arr[:, :, :, 0:60]
        out[:, :, s:s + 8, 60:62] = arr[:, :, :, 62:64]
    out += np.asarray(bias, dtype=np.float32).transpose(2, 0, 1)[None]
    return out


def _run(in_maps, trace=False, trace_cores=None):
    from concourse.bass_utils import run_bass_kernel_spmd
    nc = _get_nc()
    return run_bass_kernel_spmd(
        nc, in_maps, core_ids=list(range(NCORES)),
        trace=trace, trace_cores=trace_cores,
    )


def kernel(features, weights, bias):
    in_maps = _prep_inputs(features, weights)
    res = _run(in_maps)
    return _gather(res.results, bias)


# revision 8
# speedup vs baseline: 1.0089x; 1.0089x over previous
"""LocallyConnected2d kernel for 8 TRN2 NeuronCores (Bass/Tile).

Problem (hardcoded):
  features [32, 64, 64, 64] f32, weights [62, 62, 64, 64, 3, 3] f32,
  bias [62, 62, 64] f32 -> out [32, 64, 62, 62] f32
  out[b,o,h,w] = sum_{c,i,j} x[b,c,h+i,w+j] * W[h,w,o,c,i,j] + bias[h,w,o]

Strategy:
  - Shard over Hout: 8 cores x 8 output rows (bands [0,8,...,48,54], the last
    two overlap; host takes canonical rows from each core).
  - bf16 on the PE, fp32 PSUM accumulate. Single dual-shifted feature tile
    fA (p<64: x[c,t,w,b]; p>=64: x[c,t,w+1,b]) so a [128,32] AP slice pairs
    kernel cols (d=0,d=1) in one K=128 matmul; kernel col d=2 is covered by
    K=64 matmuls on the lower half of fA sliced at w+2 -- no second feature
    tile, saving its 5.2MB of DMA-engine traffic vs the fB variant.
  - Work unit = (half-band hg, group of 4 w): PSUM tile [128,256] with
    partitions=(4w x 32b) via col tile_position and free=(4 output rows x 64
    cout). ONE accumulation group per tile (single start=True zeroing matmul;
    per-element has_written gives overwrite-on-first-touch).
  - Matmuls grouped by stationary: a patch at absolute row t serves all
    (out-row j, kernel-row r) with j+r=t-hl in ONE matmul with a wide moving
    operand (weights host-concatenated, N up to 192). Same t-grouping for the
    d=2 (K=64) matmuls, whose weights ride a separate 64-partition stream
    (all matmul operands stay on base partition 0 -- row-64 PE tiles fault).
  - The 16 SDMA engines (~26 GB/s each, ~410 GB/s aggregate) are shared by
    all queues, and only sustain full per-packet rate when each queue walks
    ONE DRAM region strictly sequentially (a queue alternating two tensors
    was measured 2.5x slower per packet near every region switch, and it
    drags down every other concurrent stream too). So: sync ring = wr
    stream, scalar ring = featA then wd2 stream, gpsimd SWDGE = chunked
    outS stores.
  - Host: shard/pack inputs, unpack outS dumps, add bias, assemble f32 out.
"""

import numpy as np
import ml_dtypes

BF16 = ml_dtypes.bfloat16

B, CIN, COUT = 32, 64, 64
H = W = 64
HOUT = WOUT = 62
NCORES = 8
STARTS = [0, 8, 16, 24, 32, 40, 48, 54]

# t-group geometry: tau = t - hl in 0..5; valid out-rows j in [jlo, jhi]
TAUS = list(range(6))
JLO = [max(0, t - 2) for t in TAUS]
JHI = [min(3, t) for t in TAUS]
NV = [hi - lo + 1 for lo, hi in zip(JLO, JHI)]          # [1,2,3,3,2,1]
TBASE = [0]
for t in TAUS:
    TBASE.append(TBASE[-1] + 4 * NV[t] * 64)            # per-(tau) base col
WR_COLS = TBASE[-1]                                      # 3072

_STATE = {}


def _build_program():
    import concourse.tile as tile
    from concourse import bacc, mybir

    bf = mybir.dt.bfloat16
    f32 = mybir.dt.float32

    nc = bacc.Bacc(None, target_bir_lowering=False)
    featA = nc.dram_tensor("featA", [128, 10, 64, 32], bf, kind="ExternalInput")
    wr_d = nc.dram_tensor("wr", [2, 8, 128, 2, WR_COLS], bf,
                          kind="ExternalInput")
    wd2_d = nc.dram_tensor("wd2", [2, 8, 64, 2, WR_COLS], bf,
                           kind="ExternalInput")
    outS = nc.dram_tensor("outS", [2, 128, 4096], bf, kind="ExternalOutput")

    with tile.TileContext(nc) as tc:
        with tc.tile_pool(name="feat", bufs=1) as fpool, \
             tc.tile_pool(name="wr", bufs=4) as wrpool, \
             tc.tile_pool(name="wd2", bufs=4) as wdpool, \
             tc.tile_pool(name="st", bufs=2) as spool, \
             tc.tile_pool(name="ps", bufs=8, space="PSUM") as pspool:
            # featA heads the scalar ring (before the wd2 stream),
            # row-chunked so hg=0 matmuls unblock on the first chunk.
            fA = fpool.tile([128, 10, 64, 32], bf)
            nc.scalar.dma_start(fA[:, 0:6], featA[:, 0:6])
            nc.scalar.dma_start(fA[:, 6:10], featA[:, 6:10])
            # zero operands for the psum-clearing matmul (see below)
            zl = fpool.tile([1, 128], bf)
            nc.gpsimd.memset(zl[:], 0.0)
            zr = fpool.tile([1, 256], bf)
            nc.gpsimd.memset(zr[:], 0.0)
            for hg in range(2):
                hl = 4 * hg
                S = spool.tile([128, 4096], bf)
                for pr in range(8):
                    wr = wrpool.tile([128, 2, WR_COLS], bf)
                    wd2 = wdpool.tile([64, 2, WR_COLS], bf)
                    nc.sync.dma_start(wr[:], wr_d[hg, pr])
                    nc.scalar.dma_start(wd2[:], wd2_d[hg, pr])
                    for sub in range(2):
                        wg = 2 * pr + sub
                        w0 = min(4 * wg, 58)  # last group overlaps: w 58..61
                        ps = pspool.tile([128, 256], f32)
                        # K=1 zeroing matmul over the WHOLE tile: starts the
                        # accumulation group, zeroes every element, and
                        # (because its output overlaps all later MMs) forces
                        # the scheduler to keep it first; all real MMs are
                        # then pure order-free flags=0 accumulates.
                        nc.tensor.matmul(ps[:, :], zl[:], zr[:],
                                         start=True, stop=False,
                                         tile_position=(0, 0))
                        for tau in TAUS:
                            nv, jlo = NV[tau], JLO[tau]
                            for g in range(4):
                                off = TBASE[tau] + g * nv * 64
                                nc.tensor.matmul(
                                    ps[32 * g:32 * g + 32,
                                       64 * jlo:64 * (jlo + nv)],
                                    fA[:, hl + tau, w0 + g, :],
                                    wr[:, sub, off:off + nv * 64],
                                    start=False, stop=False,
                                    tile_position=(0, 32 * g),
                                )
                        # d=2 column: K=64 on the unshifted half at w+2.
                        for tau in TAUS:
                            nv, jlo = NV[tau], JLO[tau]
                            for g in range(4):
                                off = TBASE[tau] + g * nv * 64
                                nc.tensor.matmul(
                                    ps[32 * g:32 * g + 32,
                                       64 * jlo:64 * (jlo + nv)],
                                    fA[0:64, hl + tau, w0 + g + 2, :],
                                    wd2[:, sub, off:off + nv * 64],
                                    start=False, stop=(tau == 5 and g == 3),
                                    tile_position=(0, 32 * g),
                                )
                        nc.vector.tensor_copy(S[:, 256 * wg:256 * wg + 256],
                                              ps[:])
                        if wg % 8 == 7:
                            q = wg // 8
                            nc.gpsimd.dma_start(
                                outS[hg][:, 2048 * q:2048 * (q + 1)],
                                S[:, 2048 * q:2048 * (q + 1)])
    nc.compile()
    return nc


def _get_nc():
    if "nc" not in _STATE:
        _STATE["nc"] = _build_program()
    return _STATE["nc"]


def _prep_inputs(features, weights):
    """Build the 8 per-core input dicts (bf16, device layouts)."""
    x = np.asarray(features, dtype=np.float32)
    Wt = np.asarray(weights, dtype=np.float32)

    # w-slot -> real w: last group overlaps (w 58..61), no padding needed
    widx = list(range(60)) + [58, 59, 60, 61]

    in_maps = []
    for s in STARTS:
        xt = x[:, :, s:s + 10, :].transpose(1, 2, 3, 0)  # [c, 10, 64, b]
        fA = np.zeros((128, 10, 64, 32), dtype=BF16)
        fA[:64] = xt
        fA[64:, :, :63, :] = xt[:, :, 1:, :]             # w+1 shift

        Wb = Wt[s:s + 8]                                  # [8, 62, o, c, 3, 3]
        Wsel = Wb[:, widx]                                # [8, 64slots, o, c, 3, 3]
        WT = Wsel.transpose(4, 5, 3, 0, 1, 2)             # [i, jw, c, 8h, 64w, o]

        # wr: t-grouped ktiles (cells (r,0)|(r,1)); cols per (tau,g):
        #   q=0..nv-1 -> j=jlo+q, r=tau-j; value(d,c,o)=W[h,w,o,c,r,d]
        wr = np.zeros((2, 16, 128, WR_COLS), dtype=BF16)
        # wd2: t-grouped d=2 cells, K=64; value(c,o)=W[h,w,o,c,r,2]
        wd2 = np.zeros((2, 16, 64, WR_COLS), dtype=BF16)
        for tau in TAUS:
            nv, jlo = NV[tau], JLO[tau]
            view = wr[:, :, :, TBASE[tau]:TBASE[tau + 1]].reshape(
                2, 16, 128, 4, nv, 64)
            view2 = wd2[:, :, :, TBASE[tau]:TBASE[tau + 1]].reshape(
                2, 16, 64, 4, nv, 64)
            for q in range(nv):
                j = jlo + q
                r = tau - j
                for d in range(2):
                    src = WT[r, d].reshape(CIN, 2, 4, 16, 4, COUT)[:, :, j]
                    view[:, :, d * 64:(d + 1) * 64, :, q, :] = \
                        src.transpose(1, 2, 0, 3, 4)      # [hg, wg, c, g, o]
                src = WT[r, 2].reshape(CIN, 2, 4, 16, 4, COUT)[:, :, j]
                view2[:, :, :, :, q, :] = src.transpose(1, 2, 0, 3, 4)
        # two w-groups per transfer: [2, 8pair, P, 2sub, 3072]
        wr = np.ascontiguousarray(
            wr.reshape(2, 8, 2, 128, WR_COLS).transpose(0, 1, 3, 2, 4))
        wd2 = np.ascontiguousarray(
            wd2.reshape(2, 8, 2, 64, WR_COLS).transpose(0, 1, 3, 2, 4))
        in_maps.append({"featA": fA, "wr": wr, "wd2": wd2})
    return in_maps


def _gather(results, bias):
    out = np.zeros((B, COUT, HOUT, WOUT), dtype=np.float32)
    for core, s in enumerate(STARTS):
        arr = np.asarray(results[core]["outS"]).astype(np.float32)
        # [hg, g, b, wg, j, o] -> [b, o, hg, j, wg, g]
        arr = arr.reshape(2, 4, 32, 16, 4, 64).transpose(2, 5, 0, 4, 3, 1)
        arr = arr.reshape(32, 64, 8, 64)
        out[:, :, s:s + 8, 0:60] = arr[:, :, :, 0:60]
        out[:, :, s:s + 8, 60:62] = arr[:, :, :, 62:64]
    out += np.asarray(bias, dtype=np.float32).transpose(2, 0, 1)[None]
    return out


def _run(in_maps, trace=False, trace_cores=None):
    from concourse.bass_utils import run_bass_kernel_spmd
    nc = _get_nc()
    return run_bass_kernel_spmd(
        nc, in_maps, core_ids=list(range(NCORES)),
        trace=trace, trace_cores=trace_cores,
    )


def kernel(features, weights, bias):
    in_maps = _prep_inputs(features, weights)
    res = _run(in_maps)
    return _gather(res.results, bias)


# revision 13
# speedup vs baseline: 1.1246x; 1.1146x over previous
"""LocallyConnected2d kernel for 8 TRN2 NeuronCores (Bass/Tile).

Problem (hardcoded):
  features [32, 64, 64, 64] f32, weights [62, 62, 64, 64, 3, 3] f32,
  bias [62, 62, 64] f32 -> out [32, 64, 62, 62] f32
  out[b,o,h,w] = sum_{c,i,j} x[b,c,h+i,w+j] * W[h,w,o,c,i,j] + bias[h,w,o]

Strategy:
  - Shard over Hout: 8 cores x 8 output rows (bands [0,8,...,48,54], the last
    two overlap; host takes canonical rows from each core).
  - bf16 on the PE, fp32 PSUM accumulate. Contraction (c,i,j)=576 per output
    location via 14 matmuls per location-group, built on a host-baked
    "dual shifted" feature layout (partition p<64: x[c,t,w]; p>=64 carries a
    shifted copy) so a [128,32] AP slice is a ready im2col patch
    (batch = stationary cols).
  - fB (the h+1-shift dual tile for the w3 cell pair) is built ON-CHIP
    instead of loaded from HBM (-5.2MB of DMA-engine traffic): lower half is
    a vector copy of fA's lower half; upper half is a +64 partition shift of
    fA rows t+1 done with identity matmuls into PSUM at tile (0,64) (all
    operands at base partition 0 -- base-64 operands are illegal / row-64
    PE tiles fault) followed by vector PSUM->SBUF copies.
  - Work unit = (half-band hg, group of 4 w): PSUM tile [128,256] with
    partitions=(4w x 32b) via col tile_position and free=(4 output rows x 64
    cout). ONE accumulation group per tile (single start=True; per-element
    has_written gives overwrite-on-first-touch) -> no mid-tile start stalls.
  - Matmuls grouped by stationary: a patch at absolute row t serves all
    (out-row j, kernel-row r) with j+r=t-hl in ONE matmul with a wide moving
    operand (weights host-concatenated, N up to 192).
  - The 16 SDMA engines (~26 GB/s each, ~425 GB/s aggregate) only sustain
    full per-packet rate for 128-line transfers walking one DRAM region
    sequentially (64-line transfers and region-alternating queues measured
    2.5x slower per packet, dragging every concurrent stream down). So the
    weight stream is split BY HALF-BAND into two pure sequential 128-line
    streams: sync ring = featA chunk0 then hg=0 tiles; scalar ring = featA
    chunk1 then hg=1 tiles (prefetching from t=0 via its own tile pool).
    outS goes out in 4 chunked stores on the gpsimd SWDGE ring.
  - Host: shard/pack inputs, unpack outS dumps, add bias, assemble f32 out.
"""

import numpy as np
import ml_dtypes

BF16 = ml_dtypes.bfloat16

B, CIN, COUT = 32, 64, 64
H = W = 64
HOUT = WOUT = 62
NCORES = 8
STARTS = [0, 8, 16, 24, 32, 40, 48, 54]

# t-group geometry: tau = t - hl in 0..5; valid out-rows j in [jlo, jhi]
TAUS = list(range(6))
JLO = [max(0, t - 2) for t in TAUS]
JHI = [min(3, t) for t in TAUS]
NV = [hi - lo + 1 for lo, hi in zip(JLO, JHI)]          # [1,2,3,3,2,1]
TBASE = [0]
for t in TAUS:
    TBASE.append(TBASE[-1] + 4 * NV[t] * 64)            # per-(tau) base col
WR_COLS = TBASE[-1]                                      # 3072

_STATE = {}


def _build_program():
    import concourse.tile as tile
    from concourse import bacc, mybir

    bf = mybir.dt.bfloat16
    f32 = mybir.dt.float32

    nc = bacc.Bacc(None, target_bir_lowering=False)
    featA = nc.dram_tensor("featA", [128, 10, 64, 32], bf, kind="ExternalInput")
    ident = nc.dram_tensor("ident", [64, 64], bf, kind="ExternalInput")
    # wr||w3||w4(padded to 128p) merged: each (hg,wg) block is one sequential
    # 1.25MB 128-line transfer; the two hg halves stream on different rings
    wk_d = nc.dram_tensor("wk", [2, 16, 128, WR_COLS + 2048], bf,
                          kind="ExternalInput")
    outS = nc.dram_tensor("outS", [2, 128, 4096], bf, kind="ExternalOutput")

    with tile.TileContext(nc) as tc:
        with tc.tile_pool(name="feat", bufs=1) as fpool, \
             tc.tile_pool(name="wk0", bufs=5) as wk0pool, \
             tc.tile_pool(name="wk1", bufs=5) as wk1pool, \
             tc.tile_pool(name="st", bufs=2) as spool, \
             tc.tile_pool(name="ps", bufs=6, space="PSUM") as pspool, \
             tc.tile_pool(name="pt", bufs=2, space="PSUM") as ptpool:
            # featA heads the sync ring (chunk1 rides between the hg0 weight
            # tiles and is only needed for hg=1); ident is tiny.
            fA = fpool.tile([128, 10, 64, 32], bf)
            nc.sync.dma_start(fA[:, 0:6], featA[:, 0:6])
            nc.scalar.dma_start(fA[:, 6:10], featA[:, 6:10])
            ID = fpool.tile([64, 64], bf)
            nc.sync.dma_start(ID[:], ident[:])
            # zero operands for the psum-clearing matmul (see below)
            zl = fpool.tile([1, 128], bf)
            nc.gpsimd.memset(zl[:], 0.0)
            zr = fpool.tile([1, 256], bf)
            nc.gpsimd.memset(zr[:], 0.0)
            # fB[c, t, w, b] = x[c, t, w]; fB[64+c, t, w, b] = x[c, t+1, w]
            # (t = 0..7 is all w3 ever reads). Lower half: vector copy from
            # fA. Upper half: identity matmul fA[0:64, t+1] -> PSUM
            # partitions 64-127 (tile col 64), vector-copied into fB.
            fB = fpool.tile([128, 8, 64, 32], bf)

            def build_fB(t0, t1):
                nc.vector.tensor_copy(fB[0:64, t0:t1], fA[0:64, t0:t1])
                for t in range(t0, t1):
                    for c4 in range(4):
                        pt = ptpool.tile([128, 512], f32)
                        nc.tensor.matmul(
                            pt[64:128, :],
                            ID[:],
                            fA[0:64, t + 1, 16 * c4:16 * (c4 + 1), :],
                            start=True, stop=True,
                            tile_position=(0, 64),
                        )
                        nc.vector.tensor_copy(
                            fB[64:128, t, 16 * c4:16 * (c4 + 1), :],
                            pt[64:128, :])

            build_fB(0, 4)   # needs fA rows 1..4 (chunk0)
            for hg in range(2):
                hl = 4 * hg
                S = spool.tile([128, 4096], bf)
                for wg in range(16):
                    w0 = min(4 * wg, 58)   # last group overlaps: w 58..61
                    if hg == 0:
                        wk = wk0pool.tile([128, WR_COLS + 2048], bf)
                        nc.sync.dma_start(wk[:], wk_d[hg, wg])
                    else:
                        wk = wk1pool.tile([128, WR_COLS + 2048], bf)
                        nc.scalar.dma_start(wk[:], wk_d[hg, wg])
                    wr = wk[:, 0:WR_COLS]
                    w3 = wk[:, WR_COLS:WR_COLS + 1024]
                    w4 = wk[0:64, WR_COLS + 1024:WR_COLS + 2048]

                    ps = pspool.tile([128, 256], f32)
                    # K=1 zeroing matmul over the WHOLE tile: starts the
                    # accumulation group, zeroes every element, and (because
                    # its output overlaps all later MMs) forces the scheduler
                    # to keep it first; all real MMs are then pure order-free
                    # flags=0 accumulates.
                    nc.tensor.matmul(ps[:, :], zl[:], zr[:],
                                     start=True, stop=False,
                                     tile_position=(0, 0))
                    for tau in TAUS:
                        nv, jlo = NV[tau], JLO[tau]
                        for g in range(4):
                            off = TBASE[tau] + g * nv * 64
                            nc.tensor.matmul(
                                ps[32 * g:32 * g + 32,
                                   64 * jlo:64 * (jlo + nv)],
                                fA[:, hl + tau, w0 + g, :],
                                wr[:, off:off + nv * 64],
                                start=False, stop=False,
                                tile_position=(0, 32 * g),
                            )
                    for j in range(4):
                        for g in range(4):
                            off = (j * 4 + g) * 64
                            nc.tensor.matmul(
                                ps[32 * g:32 * g + 32, 64 * j:64 * j + 64],
                                fA[0:64, hl + j + 2, w0 + g + 2, :],
                                w4[:, off:off + 64],
                                start=False, stop=False,
                                tile_position=(0, 32 * g),
                            )
                    # fB-dependent matmuls last (slack for the on-chip build)
                    for j in range(4):
                        for g in range(4):
                            off = (j * 4 + g) * 64
                            nc.tensor.matmul(
                                ps[32 * g:32 * g + 32, 64 * j:64 * j + 64],
                                fB[:, hl + j, w0 + g + 2, :],
                                w3[:, off:off + 64],
                                start=False, stop=(j == 3 and g == 3),
                                tile_position=(0, 32 * g),
                            )
                    nc.vector.tensor_copy(S[:, 256 * wg:256 * wg + 256],
                                          ps[:])
                    if wg % 8 == 7:
                        q = wg // 8
                        nc.gpsimd.dma_start(
                            outS[hg][:, 2048 * q:2048 * (q + 1)],
                            S[:, 2048 * q:2048 * (q + 1)])
                if hg == 0:
                    # fB rows 4..7 for hg=1 (reads fA rows 5..8; chunk1
                    # already landed early on the scalar ring)
                    build_fB(4, 8)
    nc.compile()
    return nc


def _get_nc():
    if "nc" not in _STATE:
        _STATE["nc"] = _build_program()
    return _STATE["nc"]


def _prep_inputs(features, weights):
    """Build the 8 per-core input dicts (bf16, device layouts)."""
    x = np.asarray(features, dtype=np.float32)
    Wt = np.asarray(weights, dtype=np.float32)

    # w-slot -> real w: last group overlaps (w 58..61), no padding needed
    widx = list(range(60)) + [58, 59, 60, 61]
    ident = np.eye(64, dtype=BF16)

    in_maps = []
    for s in STARTS:
        xt = x[:, :, s:s + 10, :].transpose(1, 2, 3, 0)  # [c, 10, 64, b]
        fA = np.zeros((128, 10, 64, 32), dtype=BF16)
        fA[:64] = xt
        fA[64:, :, :63, :] = xt[:, :, 1:, :]             # w+1 shift

        Wb = Wt[s:s + 8]                                  # [8, 62, o, c, 3, 3]
        Wsel = Wb[:, widx]                                # [8, 64slots, o, c, 3, 3]
        WT = Wsel.transpose(4, 5, 3, 0, 1, 2)             # [i, jw, c, 8h, 64w, o]

        # wr: t-grouped ktiles (cells (r,0)|(r,1)); cols per (tau,g):
        #   q=0..nv-1 -> j=jlo+q, r=tau-j; value(d,c,o)=W[h,w,o,c,r,d]
        wr = np.zeros((2, 16, 128, WR_COLS), dtype=BF16)
        for tau in TAUS:
            nv, jlo = NV[tau], JLO[tau]
            view = wr[:, :, :, TBASE[tau]:TBASE[tau + 1]].reshape(
                2, 16, 128, 4, nv, 64)
            for q in range(nv):
                j = jlo + q
                r = tau - j
                for d in range(2):
                    src = WT[r, d].reshape(CIN, 2, 4, 16, 4, COUT)[:, :, j]
                    view[:, :, d * 64:(d + 1) * 64, :, q, :] = \
                        src.transpose(1, 2, 0, 3, 4)      # [hg, wg, c, g, o]
        # w3: cells (0,2) d=0 / (1,2) d=1 ; free=(j,g,o)
        w3 = np.zeros((2, 16, 128, 1024), dtype=BF16)
        for d in range(2):
            src = WT[d, 2].reshape(CIN, 2, 4, 16, 4, COUT)
            w3[:, :, d * 64:(d + 1) * 64, :] = src.transpose(
                1, 3, 0, 2, 4, 5).reshape(2, 16, 64, 1024)
        # w4: cell (2,2)
        src = WT[2, 2].reshape(CIN, 2, 4, 16, 4, COUT)
        w4 = np.ascontiguousarray(
            src.transpose(1, 3, 0, 2, 4, 5), dtype=BF16).reshape(2, 16, 64, 1024)

        w4pad = np.zeros((2, 16, 128, 1024), dtype=BF16)
        w4pad[:, :, 0:64, :] = w4
        wk = np.concatenate([wr, w3, w4pad], axis=-1)     # [2,16,128,5120]
        in_maps.append({"featA": fA, "ident": ident, "wk": wk})
    return in_maps


def _gather(results, bias):
    out = np.zeros((B, COUT, HOUT, WOUT), dtype=np.float32)
    for core, s in enumerate(STARTS):
        arr = np.asarray(results[core]["outS"]).astype(np.float32)
        # [hg, g, b, wg, j, o] -> [b, o, hg, j, wg, g]
        arr = arr.reshape(2, 4, 32, 16, 4, 64).transpose(2, 5, 0, 4, 3, 1)
        arr = arr.reshape(32, 64, 8, 64)
        out[:, :, s:s + 8, 0:60] = arr[:, :, :, 0:60]
        out[:, :, s:s + 8, 60:62] = arr[:, :, :, 62:64]
    out += np.asarray(bias, dtype=np.float32).transpose(2, 0, 1)[None]
    return out


def _run(in_maps, trace=False, trace_cores=None):
    from concourse.bass_utils import run_bass_kernel_spmd
    nc = _get_nc()
    return run_bass_kernel_spmd(
        nc, in_maps, core_ids=list(range(NCORES)),
        trace=trace, trace_cores=trace_cores,
    )


def kernel(features, weights, bias):
    in_maps = _prep_inputs(features, weights)
    res = _run(in_maps)
    return _gather(res.results, bias)


# revision 17
# speedup vs baseline: 1.2091x; 1.0751x over previous
"""LocallyConnected2d kernel for 8 TRN2 NeuronCores (Bass/Tile).

Problem (hardcoded):
  features [32, 64, 64, 64] f32, weights [62, 62, 64, 64, 3, 3] f32,
  bias [62, 62, 64] f32 -> out [32, 64, 62, 62] f32
  out[b,o,h,w] = sum_{c,i,j} x[b,c,h+i,w+j] * W[h,w,o,c,i,j] + bias[h,w,o]

Strategy:
  - Shard over Hout: 8 cores x 8 output rows (bands [0,8,...,48,54], the last
    two overlap; host takes canonical rows from each core).
  - bf16 on the PE, fp32 PSUM accumulate. Contraction (c,i,j)=576 per output
    location via 14 matmuls per location-group, built on a host-baked
    "dual shifted" feature layout (partition p<64: x[c,t,w]; p>=64 carries a
    shifted copy) so a [128,32] AP slice is a ready im2col patch
    (batch = stationary cols).
  - fB (the h+1-shift dual tile for the w3 cell pair) is built ON-CHIP
    instead of loaded from HBM (-5.2MB of DMA-engine traffic): lower half is
    a vector copy of fA's lower half; upper half is a +64 partition shift of
    fA rows t+1 done with identity matmuls into PSUM at tile (0,64) (all
    operands at base partition 0 -- base-64 operands are illegal / row-64
    PE tiles fault) followed by vector PSUM->SBUF copies. All fB work lives
    on the tensor+vector engines, off the DMA/consumption critical path.
  - Work unit = (hg, group of 4 w): PSUM tile [128,256] with partitions=
    (4w x 32b) via col tile_position and free=(4 output rows x 64 cout).
    ONE accumulation group per tile (single start=True; per-element
    has_written gives overwrite-on-first-touch) -> no mid-tile start stalls.
  - Matmuls grouped by stationary: a patch at absolute row t serves all
    (out-row j, kernel-row r) with j+r=t-hl in ONE matmul with a wide moving
    operand (weights host-concatenated, N up to 192).
  - DMA: a single HWDGE queue serializes its transfers (~340 GB/s cap) and
    the 16 SDMA engines cap at ~26 GB/s each (~425 GB/s aggregate), full
    rate only for 128-line single-region sequential streams. So the weight
    stream is split BY HALF-BAND into two pure sequential streams (sync =
    featA chunk0 + hg=0 tiles, scalar = featA chunk1 + hg=1 tiles) and the
    compute loop is WG-MAJOR, consuming (hg0,wg) and (hg1,wg) alternately so
    both queues stay consumption-unblocked and stream concurrently,
    saturating the engines. PSUM->S copies ride the scalar/Activation
    engine (keeping vector free for the fB build); outS goes out in 4
    chunked stores on the gpsimd SWDGE ring.
  - Host: shard/pack inputs, unpack outS dumps, add bias, assemble f32 out.
"""

import numpy as np
import ml_dtypes

BF16 = ml_dtypes.bfloat16

B, CIN, COUT = 32, 64, 64
H = W = 64
HOUT = WOUT = 62
NCORES = 8
STARTS = [0, 8, 16, 24, 32, 40, 48, 54]

# t-group geometry: tau = t - hl in 0..5; valid out-rows j in [jlo, jhi]
TAUS = list(range(6))
JLO = [max(0, t - 2) for t in TAUS]
JHI = [min(3, t) for t in TAUS]
NV = [hi - lo + 1 for lo, hi in zip(JLO, JHI)]          # [1,2,3,3,2,1]
TBASE = [0]
for t in TAUS:
    TBASE.append(TBASE[-1] + 4 * NV[t] * 64)            # per-(tau) base col
WR_COLS = TBASE[-1]                                      # 3072

_STATE = {}


def _build_program():
    import concourse.tile as tile
    from concourse import bacc, mybir

    bf = mybir.dt.bfloat16
    f32 = mybir.dt.float32

    nc = bacc.Bacc(None, target_bir_lowering=False)
    featA = nc.dram_tensor("featA", [128, 10, 64, 32], bf, kind="ExternalInput")
    ident = nc.dram_tensor("ident", [64, 64], bf, kind="ExternalInput")
    # wr||w3||w4(padded to 128p) merged: each (hg,wg) block is one sequential
    # 1.25MB 128-line transfer; the two hg halves stream on different rings
    wk_d = nc.dram_tensor("wk", [2, 16, 128, WR_COLS + 2048], bf,
                          kind="ExternalInput")
    outS = nc.dram_tensor("outS", [2, 128, 4096], bf, kind="ExternalOutput")

    with tile.TileContext(nc) as tc:
        with tc.tile_pool(name="feat", bufs=1) as fpool, \
             tc.tile_pool(name="wk0", bufs=5) as wk0pool, \
             tc.tile_pool(name="wk1", bufs=5) as wk1pool, \
             tc.tile_pool(name="st", bufs=2) as spool, \
             tc.tile_pool(name="ps", bufs=6, space="PSUM") as pspool, \
             tc.tile_pool(name="pt", bufs=2, space="PSUM") as ptpool:
            fA = fpool.tile([128, 10, 64, 32], bf)
            nc.sync.dma_start(fA[:, 0:6], featA[:, 0:6])
            nc.scalar.dma_start(fA[:, 6:10], featA[:, 6:10])
            ID = fpool.tile([64, 64], bf)
            nc.sync.dma_start(ID[:], ident[:])
            # zero operands for the psum-clearing matmul (see below)
            zl = fpool.tile([1, 128], bf)
            nc.gpsimd.memset(zl[:], 0.0)
            zr = fpool.tile([1, 256], bf)
            nc.gpsimd.memset(zr[:], 0.0)
            # fB[c, t, w, b] = x[c, t, w]; fB[64+c, t, w, b] = x[c, t+1, w]
            # (t = 0..7 is all w3 ever reads).
            fB = fpool.tile([128, 8, 64, 32], bf)

            def build_fB(t0, t1):
                nc.vector.tensor_copy(fB[0:64, t0:t1], fA[0:64, t0:t1])
                for t in range(t0, t1):
                    for c4 in range(4):
                        pt = ptpool.tile([128, 512], f32)
                        nc.tensor.matmul(
                            pt[64:128, :],
                            ID[:],
                            fA[0:64, t + 1, 16 * c4:16 * (c4 + 1), :],
                            start=True, stop=True,
                            tile_position=(0, 64),
                        )
                        nc.vector.tensor_copy(
                            fB[64:128, t, 16 * c4:16 * (c4 + 1), :],
                            pt[64:128, :])

            build_fB(0, 4)   # needs fA rows 1..4 (chunk0, sync ring)
            build_fB(4, 8)   # needs fA rows 5..8 (chunk1, scalar ring)

            S0 = spool.tile([128, 4096], bf)
            S1 = spool.tile([128, 4096], bf)
            Ss = [S0, S1]
            for wg in range(16):
                w0 = min(4 * wg, 58)   # last group overlaps: w 58..61
                wk0 = wk0pool.tile([128, WR_COLS + 2048], bf)
                nc.sync.dma_start(wk0[:], wk_d[0, wg])
                wk1 = wk1pool.tile([128, WR_COLS + 2048], bf)
                nc.scalar.dma_start(wk1[:], wk_d[1, wg])
                for hg, wk in ((0, wk0), (1, wk1)):
                    hl = 4 * hg
                    S = Ss[hg]
                    wr = wk[:, 0:WR_COLS]
                    w3 = wk[:, WR_COLS:WR_COLS + 1024]
                    w4 = wk[0:64, WR_COLS + 1024:WR_COLS + 2048]

                    ps = pspool.tile([128, 256], f32)
                    # K=1 zeroing matmul over the WHOLE tile: starts the
                    # accumulation group, zeroes every element, and (because
                    # its output overlaps all later MMs) forces the scheduler
                    # to keep it first; all real MMs are then pure order-free
                    # flags=0 accumulates.
                    nc.tensor.matmul(ps[:, :], zl[:], zr[:],
                                     start=True, stop=False,
                                     tile_position=(0, 0))
                    for tau in TAUS:
                        nv, jlo = NV[tau], JLO[tau]
                        for g in range(4):
                            off = TBASE[tau] + g * nv * 64
                            nc.tensor.matmul(
                                ps[32 * g:32 * g + 32,
                                   64 * jlo:64 * (jlo + nv)],
                                fA[:, hl + tau, w0 + g, :],
                                wr[:, off:off + nv * 64],
                                start=False, stop=False,
                                tile_position=(0, 32 * g),
                            )
                    for j in range(4):
                        for g in range(4):
                            off = (j * 4 + g) * 64
                            nc.tensor.matmul(
                                ps[32 * g:32 * g + 32, 64 * j:64 * j + 64],
                                fA[0:64, hl + j + 2, w0 + g + 2, :],
                                w4[:, off:off + 64],
                                start=False, stop=False,
                                tile_position=(0, 32 * g),
                            )
                    # fB-dependent matmuls last (slack for the on-chip build)
                    for j in range(4):
                        for g in range(4):
                            off = (j * 4 + g) * 64
                            nc.tensor.matmul(
                                ps[32 * g:32 * g + 32, 64 * j:64 * j + 64],
                                fB[:, hl + j, w0 + g + 2, :],
                                w3[:, off:off + 64],
                                start=False, stop=(j == 3 and g == 3),
                                tile_position=(0, 32 * g),
                            )
                    nc.scalar.copy(S[:, 256 * wg:256 * wg + 256], ps[:])
                if wg % 8 == 7:
                    q = wg // 8
                    for hg in range(2):
                        nc.gpsimd.dma_start(
                            outS[hg][:, 2048 * q:2048 * (q + 1)],
                            Ss[hg][:, 2048 * q:2048 * (q + 1)])
    nc.compile()
    return nc


def _get_nc():
    if "nc" not in _STATE:
        _STATE["nc"] = _build_program()
    return _STATE["nc"]


def _prep_inputs(features, weights):
    """Build the 8 per-core input dicts (bf16, device layouts)."""
    x = np.asarray(features, dtype=np.float32)
    Wt = np.asarray(weights, dtype=np.float32)

    # w-slot -> real w: last group overlaps (w 58..61), no padding needed
    widx = list(range(60)) + [58, 59, 60, 61]
    ident = np.eye(64, dtype=BF16)

    in_maps = []
    for s in STARTS:
        xt = x[:, :, s:s + 10, :].transpose(1, 2, 3, 0)  # [c, 10, 64, b]
        fA = np.zeros((128, 10, 64, 32), dtype=BF16)
        fA[:64] = xt
        fA[64:, :, :63, :] = xt[:, :, 1:, :]             # w+1 shift

        Wb = Wt[s:s + 8]                                  # [8, 62, o, c, 3, 3]
        Wsel = Wb[:, widx]                                # [8, 64slots, o, c, 3, 3]
        WT = Wsel.transpose(4, 5, 3, 0, 1, 2)             # [i, jw, c, 8h, 64w, o]

        # wr: t-grouped ktiles (cells (r,0)|(r,1)); cols per (tau,g):
        #   q=0..nv-1 -> j=jlo+q, r=tau-j; value(d,c,o)=W[h,w,o,c,r,d]
        wr = np.zeros((2, 16, 128, WR_COLS), dtype=BF16)
        for tau in TAUS:
            nv, jlo = NV[tau], JLO[tau]
            view = wr[:, :, :, TBASE[tau]:TBASE[tau + 1]].reshape(
                2, 16, 128, 4, nv, 64)
            for q in range(nv):
                j = jlo + q
                r = tau - j
                for d in range(2):
                    src = WT[r, d].reshape(CIN, 2, 4, 16, 4, COUT)[:, :, j]
                    view[:, :, d * 64:(d + 1) * 64, :, q, :] = \
                        src.transpose(1, 2, 0, 3, 4)      # [hg, wg, c, g, o]
        # w3: cells (0,2) d=0 / (1,2) d=1 ; free=(j,g,o)
        w3 = np.zeros((2, 16, 128, 1024), dtype=BF16)
        for d in range(2):
            src = WT[d, 2].reshape(CIN, 2, 4, 16, 4, COUT)
            w3[:, :, d * 64:(d + 1) * 64, :] = src.transpose(
                1, 3, 0, 2, 4, 5).reshape(2, 16, 64, 1024)
        # w4: cell (2,2)
        src = WT[2, 2].reshape(CIN, 2, 4, 16, 4, COUT)
        w4 = np.ascontiguousarray(
            src.transpose(1, 3, 0, 2, 4, 5), dtype=BF16).reshape(2, 16, 64, 1024)

        w4pad = np.zeros((2, 16, 128, 1024), dtype=BF16)
        w4pad[:, :, 0:64, :] = w4
        wk = np.concatenate([wr, w3, w4pad], axis=-1)     # [2,16,128,5120]
        in_maps.append({"featA": fA, "ident": ident, "wk": wk})
    return in_maps


def _gather(results, bias):
    out = np.zeros((B, COUT, HOUT, WOUT), dtype=np.float32)
    for core, s in enumerate(STARTS):
        arr = np.asarray(results[core]["outS"]).astype(np.float32)
        # [hg, g, b, wg, j, o] -> [b, o, hg, j, wg, g]
        arr = arr.reshape(2, 4, 32, 16, 4, 64).transpose(2, 5, 0, 4, 3, 1)
        arr = arr.reshape(32, 64, 8, 64)
        out[:, :, s:s + 8, 0:60] = arr[:, :, :, 0:60]
        out[:, :, s:s + 8, 60:62] = arr[:, :, :, 62:64]
    out += np.asarray(bias, dtype=np.float32).transpose(2, 0, 1)[None]
    return out


def _run(in_maps, trace=False, trace_cores=None):
    from concourse.bass_utils import run_bass_kernel_spmd
    nc = _get_nc()
    return run_bass_kernel_spmd(
        nc, in_maps, core_ids=list(range(NCORES)),
        trace=trace, trace_cores=trace_cores,
    )


def kernel(features, weights, bias):
    in_maps = _prep_inputs(features, weights)
    res = _run(in_maps)
    return _gather(res.results, bias)
